# revision 1
# baseline (speedup 1.0000x reference)
"""GCN message-passing layer (copy_src -> segment_sum -> dual degree norm)
on 8 Trainium2 NeuronCores.

Strategy (dst-sharded message passing):
  Host side (sharding/metadata only):
    - node_f = concat(u_f, v_f) * out_norm[src-side], cast to bf16.
      in-degree norm is folded into the per-edge weight w[e] = in_norm[dst[e]].
    - Edges bucketed by (core = dst range of 12500, block = 256-dst tile,
      window = 20000-src range so gather indices fit int16). Per-call chunk
      counts are variable (cross-core max) to avoid padded matmul work; each
      core's call is padded with index-0 rows to the cross-core max count so
      a static num_idxs_reg matches the generated descriptor count exactly.
  Device side (per core, one static SPMD program):
    - gpsimd: dma_gather of the 256B bf16 source-feature rows. Each bucket
      is issued as TWO half-calls on different SWDGE queues (4 queues, ucode
      pairs Q7 cores by queue), so every queue's ~1024-descriptor ring keeps
      two halves in flight and the DMA completion latency pipelines. The
      drain of ~256B random rows is the kernel's pacing item (~2.4ns/row).
    - one-hot build, split between DVE and ACT (alternating S buffers):
      DVE: S[e, slot] = w * is_equal(-iota, -slot) in one tensor_scalar
      (odd free dim 257 keeps it in a 1-port perf mode; 2-port DVE modes
      interlock with SWDGE descriptor generation). ACT: two activations,
      t = Square(iota - slot); S = Relu(w - w*t) (exact for integer iota).
      16 S buffers give the PE enough lookahead to ride out the ~5us DVE
      stalls that still occur when an op lands mid-descriptor-generation.
    - PE: psum[feat(128), slot(256)] += M[e, feat].T @ S[e, slot] in bf16,
      accumulated over a block's chunks; ACT evicts psum per block; SP DMAs
      the output.
  Host: transpose/concat the per-core [128 feat, 12544 slot] outputs.
"""

import math
from contextlib import ExitStack
from dataclasses import dataclass, field

import numpy as np

P = 128  # SBUF partitions / chunk size (edges per matmul)


def cdiv(a, b):
    return -(-a // b)


@dataclass(frozen=True)
class Cfg:
    n_nodes: int = 100000
    d: int = 128
    n_cores: int = 8
    blk: int = 256      # dst nodes per psum block (matmul N dim)
    win: int = 20000    # src window rows (must be < 32768 for int16 idxs)
    cpb: int = 8        # chunks per (block, window) bucket (set from data)
    nb_m: int = 12      # gather-destination (M tile) buffers
    nb_s: int = 16      # one-hot (S tile) buffers
    nsv: int = 8        # of the nb_s buffers, how many DVE builds (rest ACT)
    ck: tuple = ()      # per-call chunk counts (max across cores; from data)
    cmax: tuple = ()    # per-call max edge count across cores (static reg)

    @property
    def dpc(self):  # dst nodes per core
        return self.n_nodes // self.n_cores

    @property
    def nblk(self):  # blocks per core
        return cdiv(self.dpc, self.blk)

    @property
    def n_win(self):
        return cdiv(self.n_nodes, self.win)

    @property
    def ncalls(self):  # gather calls per core (= buckets per core)
        return self.nblk * self.n_win

    @property
    def nchunks(self):  # compact (unpadded) chunk count
        return sum(self.ck) if self.ck else self.ncalls * self.cpb

    @property
    def prefix(self):  # chunk index of each call's first chunk
        p = [0]
        for c in self.ck:
            p.append(p[-1] + c)
        return p

    @property
    def spc(self):  # padded idx slots per call
        return self.cpb * P


def prep_host(u_f, v_f, src, dst, cfg: Cfg | None = None):
    """Bucket/pad edges; returns (cfg, per-core input maps)."""
    import ml_dtypes

    u_f = np.asarray(u_f, dtype=np.float32)
    v_f = np.asarray(v_f, dtype=np.float32)
    src = np.asarray(src).astype(np.int64)
    dst = np.asarray(dst).astype(np.int64)
    base = cfg or Cfg()
    N, NC = base.n_nodes, base.n_cores
    E = src.shape[0]

    node_f = np.concatenate([u_f, v_f], axis=0)
    assert node_f.shape == (N, base.d)

    deg_out = np.bincount(src, minlength=N).astype(np.float32)
    deg_in = np.bincount(dst, minlength=N).astype(np.float32)
    out_norm = np.power(np.clip(deg_out, 1.0, None), np.float32(-0.5))
    in_norm = np.power(np.clip(deg_in, 1.0, None), np.float32(-0.5))
    # out-degree norm folds into the node features; in-degree norm into the
    # per-edge one-hot weight.
    node_f = np.ascontiguousarray(
        (node_f * out_norm[:, None]).astype(ml_dtypes.bfloat16)
    )
    w_edge = in_norm[dst].astype(np.float32)

    core = dst // base.dpc
    dst_loc = dst % base.dpc
    blk_id = dst_loc // base.blk
    slot = (dst_loc % base.blk).astype(np.float32)
    win_id = src // base.win
    idx16 = (src % base.win).astype(np.int16)

    nblk, W = base.nblk, base.n_win
    bucket = (core * nblk + blk_id) * W + win_id
    nbuckets = NC * nblk * W
    counts = np.bincount(bucket, minlength=nbuckets)
    # The SWDGE descriptor ring holds ~1024 descriptors; one gather call
    # per bucket requires every bucket to stay under that.
    assert counts.max() <= 1024, (
        f"bucket overflow: {counts.max()} edges > 1024; reduce cfg.win"
    )
    cpb = max(1, cdiv(int(counts.max()), P))
    ncalls = base.ncalls
    # Per-call chunk counts: max across cores so one SPMD program fits all.
    ck = np.maximum(
        1, cdiv(counts.reshape(NC, ncalls), P).max(axis=0)
    ).astype(np.int64)
    cmax = np.maximum(1, counts.reshape(NC, ncalls).max(axis=0)).astype(np.int64)
    cfg = Cfg(
        n_nodes=base.n_nodes, d=base.d, n_cores=base.n_cores, blk=base.blk,
        win=base.win, cpb=cpb, nb_m=base.nb_m, nb_s=base.nb_s, nsv=base.nsv,
        ck=tuple(int(x) for x in ck), cmax=tuple(int(x) for x in cmax),
    )
    S = cfg.spc
    nch = cfg.nchunks
    prefix = np.asarray(cfg.prefix[:-1], np.int64)

    order = np.argsort(bucket, kind="stable")
    starts = np.zeros(nbuckets + 1, np.int64)
    np.cumsum(counts, out=starts[1:])
    offs = np.arange(E, dtype=np.int64) - starts[bucket[order]]
    bo = bucket[order]
    pos = bo * S + offs  # padded layout for the gather idx stream
    # compact layout for slot/weight streams: call k's chunks start at
    # prefix[k] regardless of core (ck is the cross-core max).
    k_loc = bo % ncalls
    c_of = bo // ncalls
    pos_sw = (c_of * nch + prefix[k_loc]) * P + offs

    idx_stream = np.full(nbuckets * S, -1, np.int16)
    slot_stream = np.zeros(NC * nch * P, np.float32)
    w_stream = np.zeros(NC * nch * P, np.float32)
    idx_stream[pos] = idx16[order]
    slot_stream[pos_sw] = slot[order]
    w_stream[pos_sw] = w_edge[order]

    # Pad every core's call to the cross-core max count with index 0 (the
    # matching slot/weight entries are 0, so the extra rows are inert): the
    # static num_idxs_reg must match the post-trim descriptor count exactly
    # on every core, or the ring bookkeeping desyncs from what Q7 wrote.
    # Each call is issued as two half-calls on different queues (so each
    # queue's descriptor ring holds two in-flight halves and the completion
    # latency pipelines); the second half needs >= 1 valid idx.
    hs = (cpb // 2) * P if cpb >= 2 else 0
    cnts = counts.reshape(NC, ncalls)
    for c in range(NC):
        base_off = c * ncalls
        for k in range(ncalls):
            n0 = int(cnts[c, k])
            n1 = int(cmax[k])
            st = (base_off + k) * S
            if n0 < n1:
                idx_stream[st + n0: st + n1] = 0
            if hs and cmax[k] <= hs:
                idx_stream[st + hs] = 0

    per_core = cfg.ncalls * S
    in_maps = []
    for c in range(NC):
        seg = slice(c * per_core, (c + 1) * per_core)
        xi = idx_stream[seg].reshape(cfg.ncalls, S // 16, 16)
        xi = np.ascontiguousarray(
            np.tile(xi.transpose(2, 0, 1).reshape(16, -1), (8, 1))
        )
        seg_sw = slice(c * nch * P, (c + 1) * nch * P)
        # slots negated: ACT pass 1 computes Square(iota + bias), bias=-slot.
        sl = np.ascontiguousarray(-slot_stream[seg_sw].reshape(-1, P).T)
        wv = w_stream[seg_sw].reshape(-1, P).T
        wpos = np.ascontiguousarray(wv)
        wneg = np.ascontiguousarray(-wv)
        in_maps.append(
            {"nf": node_f, "idx": xi, "slots": sl, "wpos": wpos,
             "wneg": wneg}
        )
    return cfg, in_maps


def build_nc(cfg: Cfg):
    import concourse.bacc as bacc
    import concourse.mybir as mybir
    from concourse.library_config import mlp

    f32 = mybir.dt.float32
    bf16 = mybir.dt.bfloat16
    AF = mybir.ActivationFunctionType
    D, W, cpb, nblk = cfg.d, cfg.n_win, cfg.cpb, cfg.nblk
    ncalls, nchunks = cfg.ncalls, cfg.nchunks
    ck = cfg.ck or (cpb,) * ncalls
    prefix = cfg.prefix if cfg.ck else [cpb * k for k in range(ncalls + 1)]
    idx_cols = ncalls * cfg.spc // 16

    nc = bacc.Bacc("TRN2", target_bir_lowering=False, num_swdge_queues=4)

    nf = nc.dram_tensor("nf", [cfg.n_nodes, D], bf16, kind="ExternalInput")
    idx_d = nc.dram_tensor("idx", [P, idx_cols], mybir.dt.int16, kind="ExternalInput")
    slots_d = nc.dram_tensor("slots", [P, nchunks], f32, kind="ExternalInput")
    wpos_d = nc.dram_tensor("wpos", [P, nchunks], f32, kind="ExternalInput")
    wneg_d = nc.dram_tensor("wneg", [P, nchunks], f32, kind="ExternalInput")
    out_d = nc.dram_tensor("out", [P, nblk * cfg.blk], f32, kind="ExternalOutput")

    with ExitStack() as ctx:
        ec = ctx.enter_context
        # S/iota tiles get an odd free dim (blk+1) so DVE tensor_scalar
        # auto-detects a 1-port perf mode: 2-port DVE modes interlock with
        # SWDGE descriptor generation on the shared POOL SBUF slot and stall
        # for the remainder of the in-flight gather call.
        sfd = cfg.blk + 1
        idx_sb = ec(nc.sbuf_tensor("idx_sb", [P, idx_cols], mybir.dt.int16))
        slots_sb = ec(nc.sbuf_tensor("slots_sb", [P, nchunks], f32))
        wpos_sb = ec(nc.sbuf_tensor("wpos_sb", [P, nchunks], f32))
        wneg_sb = ec(nc.sbuf_tensor("wneg_sb", [P, nchunks], f32))
        iota_sb = ec(nc.sbuf_tensor("iota_sb", [P, sfd], bf16))
        niota_sb = ec(nc.sbuf_tensor("niota_sb", [P, sfd], bf16))
        m_sbs = [ec(nc.sbuf_tensor(f"m{j}", [P, cpb, D], bf16)) for j in range(cfg.nb_m)]
        s_sbs = [ec(nc.sbuf_tensor(f"s{j}", [P, sfd], bf16)) for j in range(cfg.nb_s)]
        t_sbs = [ec(nc.sbuf_tensor(f"t{j}", [P, sfd], bf16)) for j in range(2)]
        obufs = [ec(nc.sbuf_tensor(f"ob{j}", [P, cfg.blk], f32)) for j in range(2)]
        psums = [ec(nc.psum_tensor(f"ps{j}", [P, cfg.blk], f32)) for j in range(2)]

        io = ec(nc.semaphore("io"))
        init = ec(nc.semaphore("init"))
        gsems = [ec(nc.semaphore(f"gat{j}")) for j in range(cfg.nb_m)]
        sv = ec(nc.semaphore("sv"))    # DVE-built S chunks
        sa = ec(nc.semaphore("sa"))    # ACT-built S chunks
        pe = ec(nc.semaphore("pe"))
        ev = ec(nc.semaphore("ev"))
        osems = [ec(nc.semaphore(f"odma{j}")) for j in range(2)]

        nsv, nsa = cfg.nsv, cfg.nb_s - cfg.nsv

        def builder(t):
            """(engine, count-on-that-engine's-sem when chunk t is built).

            Buffers alternate DVE/ACT (even j -> DVE) so the two producers
            interleave at chunk granularity."""
            j = t % cfg.nb_s
            if j % 2 == 0:
                return "v", (t // cfg.nb_s) * nsv + j // 2 + 1
            return "a", (t // cfg.nb_s) * nsa + (j - 1) // 2 + 1

        with nc.Block() as block:

            @block.sync
            def _(sync):
                qc = idx_cols // 4
                for piece in range(4):
                    lo = piece * qc
                    hi = idx_cols if piece == 3 else (piece + 1) * qc
                    sync.dma_start(
                        idx_sb[:, lo:hi], idx_d[:, lo:hi]
                    ).then_inc(io, 16)
                sync.dma_start(slots_sb[:], slots_d[:]).then_inc(io, 16)
                sync.dma_start(wpos_sb[:], wpos_d[:]).then_inc(io, 16)
                sync.dma_start(wneg_sb[:], wneg_d[:]).then_inc(io, 16)
                for b in range(nblk):
                    sync.wait_ge(ev, b + 1)
                    sync.dma_start(
                        out_d[:, b * cfg.blk:(b + 1) * cfg.blk], obufs[b % 2][:]
                    ).then_inc(osems[b % 2], 16)
                sync.wait_ge(osems[0], 16 * cdiv(nblk, 2))
                if nblk > 1:
                    sync.wait_ge(osems[1], 16 * (nblk // 2))

            @block.gpsimd
            def _(g):
                g.iota(
                    iota_sb[:], [[1, sfd]], channel_multiplier=0,
                    allow_small_or_imprecise_dtypes=True,
                ).then_inc(init, 1)
                g.iota(
                    niota_sb[:], [[-1, sfd]], channel_multiplier=0,
                    allow_small_or_imprecise_dtypes=True,
                ).then_inc(init, 1)
                for j in range(cfg.nb_m):
                    g.memset(m_sbs[j][:], 0).then_inc(init, 1)
                g.load_library(mlp)
                g.wait_ge(init, 2 + cfg.nb_m)
                cmax = cfg.cmax or (cfg.spc,) * ncalls
                qc = idx_cols // 4
                io_seen = 0
                hcpb = cpb // 2
                hs = hcpb * P
                for k in range(ncalls):
                    w = k % W
                    # wait for the idx DMA piece covering this call
                    end_col = (k + 1) * cpb * 8
                    piece = 3 if end_col > 3 * qc else (end_col - 1) // qc
                    if 16 * (piece + 1) > io_seen:
                        io_seen = 16 * (piece + 1)
                        g.wait_ge(io, io_seen)
                    if k >= cfg.nb_m:
                        g.wait_ge(pe, prefix[k - cfg.nb_m + 1])
                    rows = min(cfg.win, cfg.n_nodes - w * cfg.win)
                    j = k % cfg.nb_m
                    src_v = nf[w * cfg.win: w * cfg.win + rows, :]
                    g.dma_gather(
                        m_sbs[j][:, :hcpb, :],
                        src_v,
                        idx_sb[:, k * cpb * 8: k * cpb * 8 + hcpb * 8],
                        hs,
                        min(cmax[k], hs),
                        D,
                        queue_num=(2 * k) % 4,
                    ).then_inc(gsems[j], 16)
                    g.dma_gather(
                        m_sbs[j][:, hcpb:, :],
                        src_v,
                        idx_sb[:, k * cpb * 8 + hcpb * 8:(k + 1) * cpb * 8],
                        cfg.spc - hs,
                        max(1, cmax[k] - hs),
                        D,
                        queue_num=(2 * k + 1) % 4,
                    ).then_inc(gsems[j], 16)

            @block.vector
            def _(v):
                v.wait_ge(io, 112)
                v.wait_ge(init, 2)
                for t in range(nchunks):
                    if t % cfg.nb_s % 2 != 0:
                        continue
                    if t >= cfg.nb_s:
                        v.wait_ge(pe, t - cfg.nb_s + 1)
                    v.tensor_scalar(
                        out=s_sbs[t % cfg.nb_s][:],
                        in0=niota_sb[:],
                        scalar1=slots_sb[:, t:t + 1],
                        scalar2=wpos_sb[:, t:t + 1],
                        op0=mybir.AluOpType.is_equal,
                        op1=mybir.AluOpType.mult,
                    ).then_inc(sv, 1)

            @block.scalar
            def _(a):
                a.wait_ge(io, 112)
                a.wait_ge(init, 1)
                triggers = [
                    (min(prefix[(b + 1) * W] - 1 + cfg.nb_s, nchunks - 1), b)
                    for b in range(nblk)
                ]
                triggers.reverse()  # pop from the end in ascending order

                def emit_evict(b):
                    a.wait_ge(pe, prefix[(b + 1) * W])
                    if b >= 2:
                        a.wait_ge(osems[b % 2], 16 * (b // 2))
                    a.activation(
                        obufs[b % 2][:], psums[b % 2][:], AF.Copy,
                    ).then_inc(ev, 1)

                for t in range(nchunks):
                    if t % cfg.nb_s % 2 == 0:
                        continue
                    if t >= cfg.nb_s:
                        a.wait_ge(pe, t - cfg.nb_s + 1)
                    a.activation(
                        t_sbs[t % 2][:], iota_sb[:], AF.Square,
                        bias=slots_sb[:, t:t + 1],
                    )
                    a.activation(
                        s_sbs[t % cfg.nb_s][:], t_sbs[t % 2][:], AF.Relu,
                        bias=wpos_sb[:, t:t + 1], scale=wneg_sb[:, t:t + 1],
                    ).then_inc(sa, 1)
                    while triggers and triggers[-1][0] <= t:
                        emit_evict(triggers.pop()[1])
                while triggers:
                    emit_evict(triggers.pop()[1])

            @block.tensor
            def _(te):
                t = 0
                for b in range(nblk):
                    for w in range(W):
                        k = b * W + w
                        for i in range(ck[k]):
                            if i == 0:
                                te.wait_ge(
                                    gsems[k % cfg.nb_m],
                                    32 * (k // cfg.nb_m + 1),
                                )
                            eng, cnt_needed = builder(t)
                            te.wait_ge(sv if eng == "v" else sa, cnt_needed)
                            start = (w == 0 and i == 0)
                            stop = (w == W - 1 and i == ck[k] - 1)
                            if start and b >= 2:
                                te.wait_ge(ev, b - 1)
                            te.matmul(
                                psums[b % 2][:],
                                m_sbs[k % cfg.nb_m][:, i, :],
                                s_sbs[t % cfg.nb_s][:, 0:cfg.blk],
                                start=start,
                                stop=stop,
                            ).then_inc(pe, 1)
                            t += 1

    nc.compile()
    return nc


def unshard(cfg: Cfg, results):
    out = np.empty((cfg.n_nodes, cfg.d), np.float32)
    for c in range(cfg.n_cores):
        o = results[c]["out"]
        out[c * cfg.dpc:(c + 1) * cfg.dpc, :] = o[:, :cfg.dpc].T
    return out


def run(inputs, trace=False, **spmd_kwargs):
    from concourse.bass_utils import run_bass_kernel_spmd

    cfg, in_maps = prep_host(
        inputs["u_f"], inputs["v_f"], inputs["src"], inputs["dst"]
    )
    nc = build_nc(cfg)
    res = run_bass_kernel_spmd(
        nc, in_maps, core_ids=list(range(cfg.n_cores)), trace=trace,
        **spmd_kwargs,
    )
    return unshard(cfg, res.results), res


def kernel(**inputs):
    return run(inputs)[0]



# revision 5
# speedup vs baseline: 1.1525x; 1.1525x over previous
"""GCN message-passing layer (copy_src -> segment_sum -> dual degree norm)
on 8 Trainium2 NeuronCores.

Strategy (dst-sharded message passing, v2):
  Host side (sharding/metadata only):
    - node_f = concat(u_f, v_f) * out_norm[src-side], cast to bf16.
      in-degree norm is applied to the FINAL output on the host (it is a
      per-dst-row scale, so it commutes with the edge aggregation), which
      makes the on-device one-hot matrices pure 0/1.
    - Edges bucketed by (core = dst range of 12500, block = 256-dst tile,
      window = 20000-src range so gather indices fit int16), and within a
      bucket grouped into four 64-slot sub-buckets, each padded (idx-0
      rows with slot -1) to a cross-core-max chunk count so one static
      SPMD program fits all cores. The sub-bucket with the biggest
      padding tail goes last in the call so its tail can be trimmed with
      trailing -1 indices.
  Device side (per core, one static SPMD program):
    - gpsimd: ONE dma_gather per bucket (vs 2 halves before) of the 256B
      bf16 source-feature rows -- halves the 994ns/call SWDGE ucode fixed
      cost on the Pool engine, which the trace showed 93% busy. Calls
      rotate over the 4 SWDGE queues so gen/drain pipeline.
    - DVE builds ALL one-hot tiles for a call in ONE batched tensor_tensor
      is_equal over [128, K, 66] with stride-0 broadcast APs
      (iota broadcast over chunks, per-chunk slot column broadcast over
      the 66 lanes). This replaces ~7 per-chunk tensor_scalar/activation
      builds per bucket (DVE and ACT were both ~92% busy with them) with
      one op, and frees the ACT engine entirely for psum eviction.
    - PE: psum[feat(128), 64-slot sub-range] += M[e, feat].T @ S[e, 64]
      in bf16; narrow 64-column streams cut matmul stream time 4x.
    - ACT evicts psum per 256-block; SP DMAs the output.
  Host: transpose/concat the per-core [128 feat, 12544 slot] outputs and
  scale rows by in_norm.
"""

import math
from contextlib import ExitStack
from dataclasses import dataclass, field

import numpy as np

P = 128        # SBUF partitions / chunk size (edges per matmul)
MAX_REG = 1008  # per-gather-call descriptor budget (ring holds ~1024)


def cdiv(a, b):
    return -(-a // b)


@dataclass(frozen=True)
class Cfg:
    n_nodes: int = 100000
    d: int = 128
    n_cores: int = 8
    blk: int = 256      # dst nodes per psum block
    sub: int = 64       # dst nodes per sub-bucket (matmul N dim)
    win: int = 20000    # src window rows (must be < 32768 for int16 idxs)
    sfd: int = 66       # S tile free width (>= sub+1; even for alignment)
    nb_m: int = 10      # gather-destination (M tile) buffers
    nb_s: int = 6       # one-hot (S tile) buffers
    # data-dependent schedule (cross-core maxes; baked into the program)
    ck: tuple = ()      # ck[k] = per-call tuple of per-sub chunk counts
    order: tuple = ()   # order[k] = sub visit order (biggest tail last/piece)
    regs: tuple = ()    # regs[k] = per-call tuple of per-piece descriptor counts
    pieces: tuple = ()  # pieces[k] = tuple of per-piece sub-index tuples

    @property
    def nsub(self):
        return self.blk // self.sub

    @property
    def dpc(self):  # dst nodes per core
        return self.n_nodes // self.n_cores

    @property
    def nblk(self):  # blocks per core
        return cdiv(self.dpc, self.blk)

    @property
    def n_win(self):
        return cdiv(self.n_nodes, self.win)

    @property
    def ncalls(self):  # gather buckets per core
        return self.nblk * self.n_win

    @property
    def kk(self):  # chunks per call
        return tuple(sum(c) for c in self.ck)

    @property
    def kmax(self):
        return max(self.kk)

    @property
    def nchunks(self):
        return sum(self.kk)

    @property
    def prefix(self):  # global chunk index of each call's first chunk
        p = [0]
        for c in self.kk:
            p.append(p[-1] + c)
        return p

    @property
    def idx_cols(self):  # int16 idx columns (16-wrap: 8 cols per chunk)
        return self.nchunks * 8


def prep_host(u_f, v_f, src, dst, base: Cfg | None = None):
    """Bucket/pad edges; returns (cfg, per-core input maps, in_norm)."""
    import ml_dtypes

    u_f = np.asarray(u_f, dtype=np.float32)
    v_f = np.asarray(v_f, dtype=np.float32)
    src = np.asarray(src).astype(np.int64)
    dst = np.asarray(dst).astype(np.int64)
    base = base or Cfg()
    N, NC, W = base.n_nodes, base.n_cores, base.n_win
    nblk, SUB, NS = base.nblk, base.sub, base.nsub
    ncalls = base.ncalls
    E = src.shape[0]

    node_f = np.concatenate([u_f, v_f], axis=0)
    assert node_f.shape == (N, base.d)

    deg_out = np.bincount(src, minlength=N).astype(np.float32)
    deg_in = np.bincount(dst, minlength=N).astype(np.float32)
    out_norm = np.power(np.clip(deg_out, 1.0, None), np.float32(-0.5))
    in_norm = np.power(np.clip(deg_in, 1.0, None), np.float32(-0.5))
    node_f = np.ascontiguousarray(
        (node_f * out_norm[:, None]).astype(ml_dtypes.bfloat16)
    )

    core = dst // base.dpc
    dst_loc = dst % base.dpc
    blk_id = dst_loc // base.blk
    slot256 = dst_loc % base.blk
    sub_id = slot256 // SUB
    slot_res = (slot256 % SUB).astype(np.float32)
    win_id = src // base.win
    idx16 = (src % base.win).astype(np.int16)

    k_call = blk_id * W + win_id               # call id within core
    sb = ((core * ncalls + k_call) * NS + sub_id)  # global sub-bucket id
    nsb = NC * ncalls * NS
    counts = np.bincount(sb, minlength=nsb).reshape(NC, ncalls, NS)
    cm = counts.max(axis=0)                     # [ncalls, NS]
    cm1 = np.maximum(1, cm)
    ck_arr = np.maximum(1, cdiv(cm, P))         # [ncalls, NS] chunks per sub

    # Per call: order subs so the one with the biggest trimmable tail is
    # last; split into two gather pieces when descriptors exceed the ring.
    order = []
    pieces = []
    regs = []
    for k in range(ncalls):
        tails = ck_arr[k] * P - cm1[k]
        last = int(np.argmax(tails))
        o = [s for s in range(NS) if s != last] + [last]
        full = int(ck_arr[k].sum() * P - tails[last])
        if full <= MAX_REG:
            order.append(tuple(o))
            pieces.append((tuple(o),))
            regs.append((full,))
        else:
            # split 2+2; re-pick the trailing sub inside each piece
            o2 = list(np.argsort(ck_arr[k] * P - cm1[k]))  # ascending tail
            p0 = (o2[0], o2[2]) if False else None
            # simple deterministic split: two smallest-tail subs first piece
            pa = [o2[0], o2[1]]
            pb = [o2[2], o2[3]]
            # within each piece put bigger tail last
            pa.sort(key=lambda s: tails[s])
            pb.sort(key=lambda s: tails[s])
            oo = pa + pb
            order.append(tuple(int(x) for x in oo))
            pieces.append((tuple(int(x) for x in pa), tuple(int(x) for x in pb)))
            ra = int(ck_arr[k][pa[0]] * P + cm1[k][pa[1]])
            rb = int(ck_arr[k][pb[0]] * P + cm1[k][pb[1]])
            regs.append((ra, rb))
            assert ra <= MAX_REG and rb <= MAX_REG, (k, ra, rb)

    cfg = Cfg(
        n_nodes=base.n_nodes, d=base.d, n_cores=base.n_cores, blk=base.blk,
        sub=SUB, win=base.win, sfd=base.sfd, nb_m=base.nb_m, nb_s=base.nb_s,
        ck=tuple(tuple(int(x) for x in ck_arr[k]) for k in range(ncalls)),
        order=tuple(order), regs=tuple(regs), pieces=tuple(pieces),
    )
    nch = cfg.nchunks
    prefix = cfg.prefix

    # chunk offset (within call) of each sub, following the visit order
    chunk_off = np.zeros((ncalls, NS), np.int64)
    for k in range(ncalls):
        off = 0
        for s in cfg.order[k]:
            chunk_off[k][s] = off
            off += ck_arr[k][s]

    # gathered extent per (call, sub): full chunks unless trailing in its
    # piece, where the cross-core max count suffices (tail trimmed by -1s)
    gext = ck_arr * P
    for k in range(ncalls):
        for pc in cfg.pieces[k]:
            gext[k][pc[-1]] = cm1[k][pc[-1]]

    # per-edge padded row position (within a core's nchunks*P row space)
    row_base = (np.asarray(prefix[:-1])[:, None] + chunk_off) * P  # [ncalls, NS]
    so = np.lexsort((sb,))  # stable sort edges by global sub-bucket
    sb_sorted = sb[so]
    starts = np.zeros(nsb + 1, np.int64)
    np.cumsum(counts.reshape(-1), out=starts[1:])
    offs = np.arange(E, dtype=np.int64) - starts[sb_sorted]
    c_of = sb_sorted // (ncalls * NS)
    k_of = (sb_sorted // NS) % ncalls
    s_of = sb_sorted % NS
    pos = c_of * (nch * P) + row_base[k_of, s_of] + offs

    idx_stream = np.full(NC * nch * P, -1, np.int16)
    slot_stream = np.full(NC * nch * P, -1.0, np.float32)
    idx_stream[pos] = idx16[so]
    slot_stream[pos] = slot_res[so]

    # pad gathered-but-unused rows with index 0 (slot stays -1 -> inert)
    for c in range(NC):
        cbase = c * (nch * P)
        for k in range(ncalls):
            for s in range(NS):
                n0 = int(counts[c, k, s])
                n1 = int(gext[k, s])
                if n0 < n1:
                    st = cbase + row_base[k, s]
                    idx_stream[st + n0: st + n1] = 0

    in_maps = []
    for c in range(NC):
        seg = slice(c * nch * P, (c + 1) * nch * P)
        xi = idx_stream[seg].reshape(nch * 8, 16)
        xi = np.ascontiguousarray(np.tile(xi.T, (8, 1)))
        sl = np.ascontiguousarray(
            slot_stream[seg].reshape(nch, P).T.astype(ml_dtypes.bfloat16)
        )
        in_maps.append({"nf": node_f, "idx": xi, "slots": sl})
    return cfg, in_maps, in_norm


def build_nc(cfg: Cfg):
    import concourse.bacc as bacc
    import concourse.mybir as mybir
    from concourse.ap import AP
    from concourse.library_config import mlp

    f32 = mybir.dt.float32
    bf16 = mybir.dt.bfloat16
    AF = mybir.ActivationFunctionType
    D, W, nblk, NS, SUB = cfg.d, cfg.n_win, cfg.nblk, cfg.nsub, cfg.sub
    ncalls, nchunks, kmax = cfg.ncalls, cfg.nchunks, cfg.kmax
    prefix = cfg.prefix
    idx_cols = cfg.idx_cols

    # per-call idx column offsets (8 cols per chunk, pieces contiguous)
    colpre = [p * 8 for p in prefix]

    # per-(buffer) gather-piece counts for gsems accounting
    npieces = [len(cfg.pieces[k]) for k in range(ncalls)]
    gneed = [0] * ncalls  # sem count PE must see before consuming call k
    acc = [0] * cfg.nb_m
    for k in range(ncalls):
        acc[k % cfg.nb_m] += npieces[k]
        gneed[k] = 16 * acc[k % cfg.nb_m]

    nc = bacc.Bacc("TRN2", target_bir_lowering=False, num_swdge_queues=4)

    nf = nc.dram_tensor("nf", [cfg.n_nodes, D], bf16, kind="ExternalInput")
    idx_d = nc.dram_tensor("idx", [P, idx_cols], mybir.dt.int16, kind="ExternalInput")
    slots_d = nc.dram_tensor("slots", [P, nchunks], bf16, kind="ExternalInput")
    out_d = nc.dram_tensor("out", [P, nblk * cfg.blk], f32, kind="ExternalOutput")

    with ExitStack() as ctx:
        ec = ctx.enter_context
        idx_sb = ec(nc.sbuf_tensor("idx_sb", [P, idx_cols], mybir.dt.int16))
        slots_sb = ec(nc.sbuf_tensor("slots_sb", [P, nchunks], bf16))
        iota_sb = ec(nc.sbuf_tensor("iota_sb", [P, cfg.sfd], bf16))
        m_sbs = [ec(nc.sbuf_tensor(f"m{j}", [P, kmax, D], bf16)) for j in range(cfg.nb_m)]
        s_sbs = [ec(nc.sbuf_tensor(f"s{j}", [P, kmax, cfg.sfd], bf16)) for j in range(cfg.nb_s)]
        obufs = [ec(nc.sbuf_tensor(f"ob{j}", [P, cfg.blk], f32)) for j in range(2)]
        # one PSUM BANK per (block-parity, sub): matmul start=1 resets more
        # than the addressed columns, so accumulation groups must not share
        # a bank. 2 parities x 4 subs = exactly the 8 banks.
        psums = [
            [ec(nc.psum_tensor(f"ps{j}_{s}", [P, SUB], f32)) for s in range(NS)]
            for j in range(2)
        ]

        io = ec(nc.semaphore("io"))
        init = ec(nc.semaphore("init"))
        gsems = [ec(nc.semaphore(f"gat{j}")) for j in range(cfg.nb_m)]
        sv = ec(nc.semaphore("sv"))
        pe = ec(nc.semaphore("pe"))
        ev = ec(nc.semaphore("ev"))
        osems = [ec(nc.semaphore(f"odma{j}")) for j in range(2)]

        with nc.Block() as block:

            @block.sync
            def _(sync):
                sync.dma_start(slots_sb[:], slots_d[:]).then_inc(io, 16)
                qc = idx_cols // 4
                for piece in range(4):
                    lo = piece * qc
                    hi = idx_cols if piece == 3 else (piece + 1) * qc
                    sync.dma_start(
                        idx_sb[:, lo:hi], idx_d[:, lo:hi]
                    ).then_inc(io, 16)
                for b in range(nblk):
                    sync.wait_ge(ev, b + 1)
                    sync.dma_start(
                        out_d[:, b * cfg.blk:(b + 1) * cfg.blk], obufs[b % 2][:]
                    ).then_inc(osems[b % 2], 16)
                sync.wait_ge(osems[0], 16 * cdiv(nblk, 2))
                if nblk > 1:
                    sync.wait_ge(osems[1], 16 * (nblk // 2))

            @block.gpsimd
            def _(g):
                g.iota(
                    iota_sb[:], [[1, cfg.sfd]], channel_multiplier=0,
                    allow_small_or_imprecise_dtypes=True,
                ).then_inc(init, 1)
                for j in range(cfg.nb_m):
                    g.memset(m_sbs[j][:], 0).then_inc(init, 1)
                g.load_library(mlp)
                g.wait_ge(init, 1 + cfg.nb_m)
                qc = idx_cols // 4
                io_seen = 0
                qn = 0
                for k in range(ncalls):
                    w = k % W
                    end_col = colpre[k + 1]
                    piece = 3 if end_col > 3 * qc else (end_col - 1) // qc
                    if 16 * (piece + 2) > io_seen:
                        io_seen = 16 * (piece + 2)
                        g.wait_ge(io, io_seen)
                    if k >= cfg.nb_m:
                        g.wait_ge(pe, prefix[k - cfg.nb_m + 1])
                    rows = min(cfg.win, cfg.n_nodes - w * cfg.win)
                    j = k % cfg.nb_m
                    src_v = nf[w * cfg.win: w * cfg.win + rows, :]
                    coff = 0  # chunk offset of the piece within the call
                    for pi, pc in enumerate(cfg.pieces[k]):
                        kp = sum(cfg.ck[k][s] for s in pc)
                        g.dma_gather(
                            m_sbs[j][:, coff:coff + kp, :],
                            src_v,
                            idx_sb[:, colpre[k] + coff * 8:
                                   colpre[k] + (coff + kp) * 8],
                            kp * P,
                            cfg.regs[k][pi],
                            D,
                            queue_num=qn % 4,
                        ).then_inc(gsems[j], 16)
                        qn += 1
                        coff += kp

            @block.vector
            def _(v):
                v.wait_ge(io, 16)
                v.wait_ge(init, 1)
                kk = cfg.kk
                for k in range(ncalls):
                    if k >= cfg.nb_s:
                        v.wait_ge(pe, prefix[k - cfg.nb_s + 1])
                    K = kk[k]
                    jb = k % cfg.nb_s
                    o = s_sbs[jb][:, 0:K, :]
                    a = iota_sb[:]
                    in0 = AP(a.tensor, a.offset, [list(a.ap[0]), [0, K], list(a.ap[1])])
                    b = slots_sb[:, prefix[k]:prefix[k] + K]
                    in1 = AP(b.tensor, b.offset, [list(b.ap[0]), list(b.ap[1]), [0, cfg.sfd]])
                    v.tensor_tensor(
                        o, in0, in1, mybir.AluOpType.is_equal
                    ).then_inc(sv, 1)

            @block.scalar
            def _(a):
                for b in range(nblk):
                    a.wait_ge(pe, prefix[(b + 1) * W])
                    if b >= 2:
                        a.wait_ge(osems[b % 2], 16 * (b // 2))
                    for s in range(NS):
                        ins = a.activation(
                            obufs[b % 2][:, s * SUB:(s + 1) * SUB],
                            psums[b % 2][s][:], AF.Copy,
                        )
                    ins.then_inc(ev, 1)

            @block.tensor
            def _(te):
                for b in range(nblk):
                    for w in range(W):
                        k = b * W + w
                        j = k % cfg.nb_m
                        first_of_call = True
                        t_local = 0
                        for s in cfg.order[k]:
                            ckk = cfg.ck[k][s]
                            for i in range(ckk):
                                if first_of_call:
                                    te.wait_ge(gsems[j], gneed[k])
                                    te.wait_ge(sv, k + 1)
                                    if w == 0 and b >= 2:
                                        te.wait_ge(ev, b - 1)
                                    first_of_call = False
                                start = (w == 0 and i == 0)
                                stop = (w == W - 1 and i == ckk - 1)
                                te.matmul(
                                    psums[b % 2][s][:],
                                    m_sbs[j][:, t_local, :],
                                    s_sbs[k % cfg.nb_s][:, t_local, 0:SUB],
                                    start=start,
                                    stop=stop,
                                ).then_inc(pe, 1)
                                t_local += 1

    nc.compile()
    return nc


def unshard(cfg: Cfg, results, in_norm):
    out = np.empty((cfg.n_nodes, cfg.d), np.float32)
    for c in range(cfg.n_cores):
        o = results[c]["out"]
        out[c * cfg.dpc:(c + 1) * cfg.dpc, :] = o[:, :cfg.dpc].T
    out *= in_norm[:, None]
    return out


def run(inputs, trace=False, **spmd_kwargs):
    from concourse.bass_utils import run_bass_kernel_spmd

    cfg, in_maps, in_norm = prep_host(
        inputs["u_f"], inputs["v_f"], inputs["src"], inputs["dst"]
    )
    nc = build_nc(cfg)
    res = run_bass_kernel_spmd(
        nc, in_maps, core_ids=list(range(cfg.n_cores)), trace=trace,
        **spmd_kwargs,
    )
    return unshard(cfg, res.results, in_norm), res


def kernel(**inputs):
    return run(inputs)[0]


# revision 6
# speedup vs baseline: 1.3538x; 1.1747x over previous
"""GCN message-passing layer (copy_src -> segment_sum -> dual degree norm)
on 8 Trainium2 NeuronCores.

Strategy (dst-sharded message passing, v2):
  Host side (sharding/metadata only):
    - node_f = concat(u_f, v_f) * out_norm[src-side], cast to bf16.
      in-degree norm is applied to the FINAL output on the host (it is a
      per-dst-row scale, so it commutes with the edge aggregation), which
      makes the on-device one-hot matrices pure 0/1.
    - Edges bucketed by (core = dst range of 12500, block = 256-dst tile,
      window = 20000-src range so gather indices fit int16), and within a
      bucket grouped into four 64-slot sub-buckets, each padded (idx-0
      rows with slot -1) to a cross-core-max chunk count so one static
      SPMD program fits all cores. The sub-bucket with the biggest
      padding tail goes last in the call so its tail can be trimmed with
      trailing -1 indices.
  Device side (per core, one static SPMD program):
    - gpsimd: ONE dma_gather per bucket (vs 2 halves before) of the 256B
      bf16 source-feature rows -- halves the 994ns/call SWDGE ucode fixed
      cost on the Pool engine, which the trace showed 93% busy. Calls
      rotate over the 4 SWDGE queues so gen/drain pipeline.
    - DVE builds ALL one-hot tiles for a call in ONE batched tensor_tensor
      is_equal over [128, K, 66] with stride-0 broadcast APs
      (iota broadcast over chunks, per-chunk slot column broadcast over
      the 66 lanes). This replaces ~7 per-chunk tensor_scalar/activation
      builds per bucket (DVE and ACT were both ~92% busy with them) with
      one op, and frees the ACT engine entirely for psum eviction.
    - PE: psum[feat(128), 64-slot sub-range] += M[e, feat].T @ S[e, 64]
      in bf16; narrow 64-column streams cut matmul stream time 4x.
    - ACT evicts psum per 256-block; SP DMAs the output.
  Host: transpose/concat the per-core [128 feat, 12544 slot] outputs and
  scale rows by in_norm.
"""

import math
from contextlib import ExitStack
from dataclasses import dataclass, field

import numpy as np

P = 128        # SBUF partitions / chunk size (edges per matmul)
MAX_REG = 1008  # per-gather-call descriptor budget (ring holds ~1024)


def cdiv(a, b):
    return -(-a // b)


@dataclass(frozen=True)
class Cfg:
    n_nodes: int = 100000
    d: int = 128
    n_cores: int = 8
    blk: int = 256      # dst nodes per psum block
    sub: int = 64       # dst nodes per sub-bucket (matmul N dim)
    win: int = 20000    # src window rows (must be < 32768 for int16 idxs)
    sfd: int = 66       # S tile free width (>= sub+1; even for alignment)
    nb_m: int = 10      # gather-destination (M tile) buffers
    nb_s: int = 6       # one-hot (S tile) buffers
    # data-dependent schedule (cross-core maxes; baked into the program)
    ck: tuple = ()      # ck[k] = per-call tuple of per-sub chunk counts
    order: tuple = ()   # order[k] = sub visit order (biggest tail last/piece)
    regs: tuple = ()    # regs[k] = per-call tuple of per-piece descriptor counts
    pieces: tuple = ()  # pieces[k] = tuple of per-piece sub-index tuples

    @property
    def nsub(self):
        return self.blk // self.sub

    @property
    def dpc(self):  # dst nodes per core
        return self.n_nodes // self.n_cores

    @property
    def nblk(self):  # blocks per core
        return cdiv(self.dpc, self.blk)

    @property
    def n_win(self):
        return cdiv(self.n_nodes, self.win)

    @property
    def ncalls(self):  # gather buckets per core
        return self.nblk * self.n_win

    @property
    def kk(self):  # chunks per call
        return tuple(sum(c) for c in self.ck)

    @property
    def kmax(self):
        return max(self.kk)

    @property
    def nchunks(self):
        return sum(self.kk)

    @property
    def prefix(self):  # global chunk index of each call's first chunk
        p = [0]
        for c in self.kk:
            p.append(p[-1] + c)
        return p

    @property
    def idx_cols(self):  # int16 idx columns (16-wrap: 8 cols per chunk)
        return self.nchunks * 8


def prep_host(u_f, v_f, src, dst, base: Cfg | None = None):
    """Bucket/pad edges; returns (cfg, per-core input maps, in_norm)."""
    import ml_dtypes

    u_f = np.asarray(u_f, dtype=np.float32)
    v_f = np.asarray(v_f, dtype=np.float32)
    src = np.asarray(src).astype(np.int64)
    dst = np.asarray(dst).astype(np.int64)
    base = base or Cfg()
    N, NC, W = base.n_nodes, base.n_cores, base.n_win
    nblk, SUB, NS = base.nblk, base.sub, base.nsub
    ncalls = base.ncalls
    E = src.shape[0]

    node_f = np.concatenate([u_f, v_f], axis=0)
    assert node_f.shape == (N, base.d)

    deg_out = np.bincount(src, minlength=N).astype(np.float32)
    deg_in = np.bincount(dst, minlength=N).astype(np.float32)
    out_norm = np.power(np.clip(deg_out, 1.0, None), np.float32(-0.5))
    in_norm = np.power(np.clip(deg_in, 1.0, None), np.float32(-0.5))
    node_f = np.ascontiguousarray(
        (node_f * out_norm[:, None]).astype(ml_dtypes.bfloat16)
    )

    core = dst // base.dpc
    dst_loc = dst % base.dpc
    blk_id = dst_loc // base.blk
    slot256 = dst_loc % base.blk
    sub_id = slot256 // SUB
    slot_res = (slot256 % SUB).astype(np.float32)
    win_id = src // base.win
    idx16 = (src % base.win).astype(np.int16)

    k_call = blk_id * W + win_id               # call id within core
    sb = ((core * ncalls + k_call) * NS + sub_id)  # global sub-bucket id
    nsb = NC * ncalls * NS
    counts = np.bincount(sb, minlength=nsb).reshape(NC, ncalls, NS)
    cm = counts.max(axis=0)                     # [ncalls, NS]
    cm1 = np.maximum(1, cm)
    ck_arr = np.maximum(1, cdiv(cm, P))         # [ncalls, NS] chunks per sub

    # Per call: order subs so the one with the biggest trimmable tail is
    # last; split into two gather pieces when descriptors exceed the ring.
    order = []
    pieces = []
    regs = []
    for k in range(ncalls):
        tails = ck_arr[k] * P - cm1[k]
        last = int(np.argmax(tails))
        o = [s for s in range(NS) if s != last] + [last]
        full = int(ck_arr[k].sum() * P - tails[last])
        if full <= MAX_REG:
            order.append(tuple(o))
            pieces.append((tuple(o),))
            regs.append((full,))
        else:
            # split 2+2; re-pick the trailing sub inside each piece
            o2 = list(np.argsort(ck_arr[k] * P - cm1[k]))  # ascending tail
            p0 = (o2[0], o2[2]) if False else None
            # simple deterministic split: two smallest-tail subs first piece
            pa = [o2[0], o2[1]]
            pb = [o2[2], o2[3]]
            # within each piece put bigger tail last
            pa.sort(key=lambda s: tails[s])
            pb.sort(key=lambda s: tails[s])
            oo = pa + pb
            order.append(tuple(int(x) for x in oo))
            pieces.append((tuple(int(x) for x in pa), tuple(int(x) for x in pb)))
            ra = int(ck_arr[k][pa[0]] * P + cm1[k][pa[1]])
            rb = int(ck_arr[k][pb[0]] * P + cm1[k][pb[1]])
            regs.append((ra, rb))
            assert ra <= MAX_REG and rb <= MAX_REG, (k, ra, rb)

    cfg = Cfg(
        n_nodes=base.n_nodes, d=base.d, n_cores=base.n_cores, blk=base.blk,
        sub=SUB, win=base.win, sfd=base.sfd, nb_m=base.nb_m, nb_s=base.nb_s,
        ck=tuple(tuple(int(x) for x in ck_arr[k]) for k in range(ncalls)),
        order=tuple(order), regs=tuple(regs), pieces=tuple(pieces),
    )
    nch = cfg.nchunks
    prefix = cfg.prefix

    # chunk offset (within call) of each sub, following the visit order
    chunk_off = np.zeros((ncalls, NS), np.int64)
    for k in range(ncalls):
        off = 0
        for s in cfg.order[k]:
            chunk_off[k][s] = off
            off += ck_arr[k][s]

    # gathered extent per (call, sub): full chunks unless trailing in its
    # piece, where the cross-core max count suffices (tail trimmed by -1s)
    gext = ck_arr * P
    for k in range(ncalls):
        for pc in cfg.pieces[k]:
            gext[k][pc[-1]] = cm1[k][pc[-1]]

    # per-edge padded row position (within a core's nchunks*P row space)
    row_base = (np.asarray(prefix[:-1])[:, None] + chunk_off) * P  # [ncalls, NS]
    so = np.lexsort((sb,))  # stable sort edges by global sub-bucket
    sb_sorted = sb[so]
    starts = np.zeros(nsb + 1, np.int64)
    np.cumsum(counts.reshape(-1), out=starts[1:])
    offs = np.arange(E, dtype=np.int64) - starts[sb_sorted]
    c_of = sb_sorted // (ncalls * NS)
    k_of = (sb_sorted // NS) % ncalls
    s_of = sb_sorted % NS
    pos = c_of * (nch * P) + row_base[k_of, s_of] + offs

    idx_stream = np.full(NC * nch * P, -1, np.int16)
    slot_stream = np.full(NC * nch * P, -1.0, np.float32)
    idx_stream[pos] = idx16[so]
    slot_stream[pos] = slot_res[so]

    # pad gathered-but-unused rows with index 0 (slot stays -1 -> inert)
    for c in range(NC):
        cbase = c * (nch * P)
        for k in range(ncalls):
            for s in range(NS):
                n0 = int(counts[c, k, s])
                n1 = int(gext[k, s])
                if n0 < n1:
                    st = cbase + row_base[k, s]
                    idx_stream[st + n0: st + n1] = 0

    in_maps = []
    for c in range(NC):
        seg = slice(c * nch * P, (c + 1) * nch * P)
        xi = idx_stream[seg].reshape(nch * 8, 16)
        xi = np.ascontiguousarray(np.tile(xi.T, (8, 1)))
        sl = np.ascontiguousarray(
            slot_stream[seg].reshape(nch, P).T.astype(ml_dtypes.bfloat16)
        )
        in_maps.append({"nf": node_f, "idx": xi, "slots": sl})
    return cfg, in_maps, in_norm


def build_nc(cfg: Cfg):
    import concourse.bacc as bacc
    import concourse.mybir as mybir
    from concourse.ap import AP
    from concourse.library_config import mlp

    f32 = mybir.dt.float32
    bf16 = mybir.dt.bfloat16
    AF = mybir.ActivationFunctionType
    D, W, nblk, NS, SUB = cfg.d, cfg.n_win, cfg.nblk, cfg.nsub, cfg.sub
    ncalls, nchunks, kmax = cfg.ncalls, cfg.nchunks, cfg.kmax
    prefix = cfg.prefix
    idx_cols = cfg.idx_cols

    # per-call idx column offsets (8 cols per chunk, pieces contiguous)
    colpre = [p * 8 for p in prefix]

    # per-(buffer) gather-piece counts for gsems accounting
    npieces = [len(cfg.pieces[k]) for k in range(ncalls)]
    gneed = [0] * ncalls  # sem count PE must see before consuming call k
    acc = [0] * cfg.nb_m
    for k in range(ncalls):
        acc[k % cfg.nb_m] += npieces[k]
        gneed[k] = 16 * acc[k % cfg.nb_m]

    nc = bacc.Bacc(
        "TRN2", target_bir_lowering=False, num_swdge_queues=4,
        dynamic_dma_scratch_size=49152,
    )

    nf = nc.dram_tensor("nf", [cfg.n_nodes, D], bf16, kind="ExternalInput")
    idx_d = nc.dram_tensor("idx", [P, idx_cols], mybir.dt.int16, kind="ExternalInput")
    slots_d = nc.dram_tensor("slots", [P, nchunks], bf16, kind="ExternalInput")
    out_d = nc.dram_tensor("out", [P, nblk * cfg.blk], f32, kind="ExternalOutput")

    with ExitStack() as ctx:
        ec = ctx.enter_context
        idx_sb = ec(nc.sbuf_tensor("idx_sb", [P, idx_cols], mybir.dt.int16))
        slots_sb = ec(nc.sbuf_tensor("slots_sb", [P, nchunks], bf16))
        iota_sb = ec(nc.sbuf_tensor("iota_sb", [P, cfg.sfd], bf16))
        m_sbs = [ec(nc.sbuf_tensor(f"m{j}", [P, kmax, D], bf16)) for j in range(cfg.nb_m)]
        s_sbs = [ec(nc.sbuf_tensor(f"s{j}", [P, kmax, cfg.sfd], bf16)) for j in range(cfg.nb_s)]
        obufs = [ec(nc.sbuf_tensor(f"ob{j}", [P, cfg.blk], f32)) for j in range(2)]
        # one PSUM BANK per (block-parity, sub): matmul start=1 resets more
        # than the addressed columns, so accumulation groups must not share
        # a bank. 2 parities x 4 subs = exactly the 8 banks.
        psums = [
            [ec(nc.psum_tensor(f"ps{j}_{s}", [P, SUB], f32)) for s in range(NS)]
            for j in range(2)
        ]

        io = ec(nc.semaphore("io"))
        init = ec(nc.semaphore("init"))
        gsems = [ec(nc.semaphore(f"gat{j}")) for j in range(cfg.nb_m)]
        sv = ec(nc.semaphore("sv"))
        pe = ec(nc.semaphore("pe"))
        ev = ec(nc.semaphore("ev"))
        osems = [ec(nc.semaphore(f"odma{j}")) for j in range(2)]

        with nc.Block() as block:

            @block.sync
            def _(sync):
                sync.dma_start(slots_sb[:], slots_d[:]).then_inc(io, 16)
                qc = idx_cols // 4
                for piece in range(4):
                    lo = piece * qc
                    hi = idx_cols if piece == 3 else (piece + 1) * qc
                    sync.dma_start(
                        idx_sb[:, lo:hi], idx_d[:, lo:hi]
                    ).then_inc(io, 16)
                for b in range(nblk):
                    sync.wait_ge(ev, b + 1)
                    sync.dma_start(
                        out_d[:, b * cfg.blk:(b + 1) * cfg.blk], obufs[b % 2][:]
                    ).then_inc(osems[b % 2], 16)
                sync.wait_ge(osems[0], 16 * cdiv(nblk, 2))
                if nblk > 1:
                    sync.wait_ge(osems[1], 16 * (nblk // 2))

            @block.gpsimd
            def _(g):
                g.iota(
                    iota_sb[:], [[1, cfg.sfd]], channel_multiplier=0,
                    allow_small_or_imprecise_dtypes=True,
                ).then_inc(init, 1)
                for j in range(cfg.nb_m):
                    g.memset(m_sbs[j][:], 0).then_inc(init, 1)
                g.load_library(mlp)
                g.wait_ge(init, 1 + cfg.nb_m)
                qc = idx_cols // 4
                io_seen = 0
                qn = 0
                for k in range(ncalls):
                    w = k % W
                    end_col = colpre[k + 1]
                    piece = 3 if end_col > 3 * qc else (end_col - 1) // qc
                    if 16 * (piece + 2) > io_seen:
                        io_seen = 16 * (piece + 2)
                        g.wait_ge(io, io_seen)
                    if k >= cfg.nb_m:
                        g.wait_ge(pe, prefix[k - cfg.nb_m + 1])
                    rows = min(cfg.win, cfg.n_nodes - w * cfg.win)
                    j = k % cfg.nb_m
                    src_v = nf[w * cfg.win: w * cfg.win + rows, :]
                    coff = 0  # chunk offset of the piece within the call
                    for pi, pc in enumerate(cfg.pieces[k]):
                        kp = sum(cfg.ck[k][s] for s in pc)
                        g.dma_gather(
                            m_sbs[j][:, coff:coff + kp, :],
                            src_v,
                            idx_sb[:, colpre[k] + coff * 8:
                                   colpre[k] + (coff + kp) * 8],
                            kp * P,
                            cfg.regs[k][pi],
                            D,
                            queue_num=qn % 4,
                        ).then_inc(gsems[j], 16)
                        qn += 1
                        coff += kp

            @block.vector
            def _(v):
                v.wait_ge(io, 16)
                v.wait_ge(init, 1)
                kk = cfg.kk
                for k in range(ncalls):
                    if k >= cfg.nb_s:
                        v.wait_ge(pe, prefix[k - cfg.nb_s + 1])
                    K = kk[k]
                    jb = k % cfg.nb_s
                    o = s_sbs[jb][:, 0:K, :]
                    a = iota_sb[:]
                    in0 = AP(a.tensor, a.offset, [list(a.ap[0]), [0, K], list(a.ap[1])])
                    b = slots_sb[:, prefix[k]:prefix[k] + K]
                    in1 = AP(b.tensor, b.offset, [list(b.ap[0]), list(b.ap[1]), [0, cfg.sfd]])
                    v.tensor_tensor(
                        o, in0, in1, mybir.AluOpType.is_equal
                    ).then_inc(sv, 1)

            @block.scalar
            def _(a):
                for b in range(nblk):
                    a.wait_ge(pe, prefix[(b + 1) * W])
                    if b >= 2:
                        a.wait_ge(osems[b % 2], 16 * (b // 2))
                    for s in range(NS):
                        ins = a.activation(
                            obufs[b % 2][:, s * SUB:(s + 1) * SUB],
                            psums[b % 2][s][:], AF.Copy,
                        )
                    ins.then_inc(ev, 1)

            @block.tensor
            def _(te):
                for b in range(nblk):
                    for w in range(W):
                        k = b * W + w
                        j = k % cfg.nb_m
                        first_of_call = True
                        t_local = 0
                        for s in cfg.order[k]:
                            ckk = cfg.ck[k][s]
                            for i in range(ckk):
                                if first_of_call:
                                    te.wait_ge(gsems[j], gneed[k])
                                    te.wait_ge(sv, k + 1)
                                    if w == 0 and b >= 2:
                                        te.wait_ge(ev, b - 1)
                                    first_of_call = False
                                start = (w == 0 and i == 0)
                                stop = (w == W - 1 and i == ckk - 1)
                                te.matmul(
                                    psums[b % 2][s][:],
                                    m_sbs[j][:, t_local, :],
                                    s_sbs[k % cfg.nb_s][:, t_local, 0:SUB],
                                    start=start,
                                    stop=stop,
                                ).then_inc(pe, 1)
                                t_local += 1

    nc.compile()
    return nc


def unshard(cfg: Cfg, results, in_norm):
    out = np.empty((cfg.n_nodes, cfg.d), np.float32)
    for c in range(cfg.n_cores):
        o = results[c]["out"]
        out[c * cfg.dpc:(c + 1) * cfg.dpc, :] = o[:, :cfg.dpc].T
    out *= in_norm[:, None]
    return out


def run(inputs, trace=False, **spmd_kwargs):
    from concourse.bass_utils import run_bass_kernel_spmd

    cfg, in_maps, in_norm = prep_host(
        inputs["u_f"], inputs["v_f"], inputs["src"], inputs["dst"]
    )
    nc = build_nc(cfg)
    res = run_bass_kernel_spmd(
        nc, in_maps, core_ids=list(range(cfg.n_cores)), trace=trace,
        **spmd_kwargs,
    )
    return unshard(cfg, res.results, in_norm), res


def kernel(**inputs):
    return run(inputs)[0]
